# revision 9
# baseline (speedup 1.0000x reference)
"""Masked graph-attention aggregator on 8 Trainium2 NeuronCores (Bass/Tile).

Computation (the nn.Module this implements):
    q/k/v = x @ W{q,k,v}.T + b                     -> [H=8, N=4096, DH=32]
    att   = softmax(mask(q k^T / sqrt(DH)))        mask from edge_index
    y     = att @ v                                -> [N, 256]
    out   = concat([x, y], 1) @ Wp.T + bp          -> [N, 256]

Sharding: query rows split 512 per core (8 cores); x and weights replicated;
each core gets the edge subset whose source (query) node falls in its rows,
grouped by key into padded per-key lists (a pure relayout of the edge-index
input).  Output rows are concatenated on host.

Per-core device pipeline:
  - adjacency mask^T [4096 keys, 512 queries] built in SBUF by GPSIMD
    local_scatter from the per-key lists; the device subtracts the row
    offset, drops out-of-range/padded entries, and drops duplicate edges
    (adjacent after the host's sort) with a shifted-compare.
  - x^T via bf16 cast + DMA xbar transpose; W^T via PE identity transposes.
  - Q^T/K^T head-major (d on partitions) bf16; V key-major bf16 with a ones
    column per head riding along to produce softmax denominators.
  - attention per head-pair, per 128-key chunk: S^T[k,q] = K Q^T on PE
    (row-tiled, contract=32), exp on ACT with the 1/sqrt(DH) scale folded
    into its free affine (fp32 PSUM input), mask multiply on DVE (bf16 2x),
    PV accumulation with lhsT=[V_h | 1] giving [y^T_h ; Z_h] in PSUM.
  - y^T scaled by 1/Z (fast-accurate DVE reciprocal + PE rank-1 broadcast),
    packed via SBUF-to-SBUF DMA; final projection in fp32 with the bias as
    a rank-1 PE accumulation.
"""

import numpy as np

import concourse.bass as bass
import concourse.mybir as mybir
import concourse.tile as tile
from concourse import library_config
from concourse.library_overlay import lower_extended_insts

N = 4096
D = 256
H = 8
DH = 32
NCORES = 8
QR = N // NCORES  # 512 query rows per core
W = 24            # per-key edge-list width (max in-shard degree of a key)
SCALE = 1.0 / float(np.sqrt(np.float32(DH)))

f32 = mybir.dt.float32
bf16 = mybir.dt.bfloat16
i16 = mybir.dt.int16

AF = mybir.ActivationFunctionType
OP = mybir.AluOpType


def _split_multi_waits(nc):
    """This toolchain's walrus encodes at most ONE sync-wait per
    instruction; TileContext can attach several (e.g. the final drain).
    Move extras onto single-wait NoOps inserted before the instruction on
    the same engine."""
    ctr = 0
    for f in nc.m.functions:
        for bb in f.blocks:
            il = bb.instructions
            i = 0
            while i < len(il):
                ins = il[i]
                si = ins.sync_info
                if si is not None and len(si.on_wait) > 1:
                    waits = list(si.on_wait)
                    ins.sync_info = mybir.SyncInfo(
                        on_wait=[waits[-1]], on_update=list(si.on_update)
                    )
                    for w in waits[:-1]:
                        ctr += 1
                        nop = mybir.InstNoOp(
                            name=f"I-waitsplit-{ctr}", ins=[], outs=[]
                        )
                        nop.engine = ins.engine
                        nop.sync_info = mybir.SyncInfo(on_wait=[w], on_update=[])
                        il.insert(i, nop)
                        i += 1
                i += 1


def build_program(split: bool = True) -> bass.Bass:
    nc = bass.Bass()

    x_full = nc.dram_tensor("x_full", [N, D], f32, kind="ExternalInput")
    x_rows = nc.dram_tensor("x_rows", [QR, D], f32, kind="ExternalInput")
    klists = nc.dram_tensor("klists", [N, W], i16, kind="ExternalInput")
    q0v = nc.dram_tensor("q0v", [128, 1], f32, kind="ExternalInput")
    ident_in = nc.dram_tensor("ident_in", [128, 128], f32, kind="ExternalInput")
    sel_in = nc.dram_tensor("sel_in", [H, H * 32], f32, kind="ExternalInput")
    Wq = nc.dram_tensor("Wq", [D, D], f32, kind="ExternalInput")
    Wk = nc.dram_tensor("Wk", [D, D], f32, kind="ExternalInput")
    Wv = nc.dram_tensor("Wv", [D, D], f32, kind="ExternalInput")
    bq = nc.dram_tensor("bq", [D, 1], f32, kind="ExternalInput")
    bk = nc.dram_tensor("bk", [D, 1], f32, kind="ExternalInput")
    bv = nc.dram_tensor("bv", [1, D], f32, kind="ExternalInput")
    Wp = nc.dram_tensor("Wp", [D, 2 * D], f32, kind="ExternalInput")
    bp = nc.dram_tensor("bp", [1, D], f32, kind="ExternalInput")
    out = nc.dram_tensor("out", [QR, D], f32, kind="ExternalOutput")
    xbf_dram = nc.dram_tensor("xbf_dram", [N, D], bf16)

    with tile.TileContext(nc) as tc:
        with (
            tc.tile_pool(name="cons", bufs=1) as cons,
            tc.tile_pool(name="big", bufs=1) as big,
            tc.tile_pool(name="work", bufs=3) as work,
        ):
            nc.gpsimd.load_library(library_config.local_scatter)

            # ---------- constants ----------
            ident = cons.tile([128, 128], f32)
            nc.sync.dma_start(out=ident[:], in_=ident_in[:])
            ones_bf_1x128 = cons.tile([1, 128], bf16)
            nc.vector.memset(ones_bf_1x128[:], 1.0)
            ones_f_128x128 = cons.tile([128, 128], f32)
            nc.vector.memset(ones_f_128x128[:], 1.0)
            ones_w = cons.tile([128, W], bf16)
            nc.vector.memset(ones_w[:], 1.0)
            q0t = cons.tile([128, 1], f32)
            nc.sync.dma_start(out=q0t[:], in_=q0v[:])

            bq_sb = cons.tile([128, 2], f32)
            bk_sb = cons.tile([128, 2], f32)
            for dt_ in range(2):
                nc.sync.dma_start(
                    out=bq_sb[:, dt_ : dt_ + 1], in_=bq[dt_ * 128 : (dt_ + 1) * 128, :]
                )
                nc.sync.dma_start(
                    out=bk_sb[:, dt_ : dt_ + 1], in_=bk[dt_ * 128 : (dt_ + 1) * 128, :]
                )
            bv_f = cons.tile([1, D], f32)
            nc.sync.dma_start(out=bv_f[:], in_=bv[:])
            bv_bf = cons.tile([1, D], bf16)
            nc.vector.tensor_copy(bv_bf[:], bv_f[:])
            bp_sb = cons.tile([1, D], f32)
            nc.sync.dma_start(out=bp_sb[:], in_=bp[:])

            # ---------- mask build (GPSIMD local_scatter) ----------
            kl = big.tile([128, 32 * W], i16)
            for g in range(8):
                nc.sync.dma_start(
                    out=kl[:, g * 4 * W : (g + 1) * 4 * W].rearrange(
                        "p (b w) -> p b w", b=4
                    ),
                    in_=klists[g * 512 : (g + 1) * 512, :].rearrange(
                        "(b p) w -> p b w", p=128
                    ),
                )
            klf = big.tile([128, 32 * W], f32)
            nc.vector.tensor_copy(klf[:], kl[:])
            dup = big.tile([128, 32 * W], f32)
            nc.vector.memset(dup[:], 0.0)
            nc.vector.tensor_tensor(
                out=dup[:, 1:], in0=klf[:, 1:], in1=klf[:, : 32 * W - 1],
                op=OP.is_equal,
            )
            segmask = cons.tile([128, 32 * W], f32)
            nc.vector.memset(segmask[:], 1.0)
            nc.vector.memset(
                segmask[:].rearrange("p (s w) -> p s w", w=W)[:, :, 0:1], 0.0
            )
            nc.vector.tensor_tensor(out=dup[:], in0=dup[:], in1=segmask[:], op=OP.mult)
            nc.vector.tensor_scalar(
                out=klf[:], in0=klf[:], scalar1=q0t[:], scalar2=None, op0=OP.subtract
            )
            g1 = big.tile([128, 32 * W], f32)
            nc.vector.tensor_scalar(
                out=g1[:], in0=klf[:], scalar1=0.0, scalar2=None, op0=OP.is_ge
            )
            g2 = big.tile([128, 32 * W], f32)
            nc.vector.tensor_scalar(
                out=g2[:], in0=klf[:], scalar1=float(QR), scalar2=None, op0=OP.is_lt
            )
            nc.vector.tensor_tensor(out=g1[:], in0=g1[:], in1=g2[:], op=OP.mult)
            nc.vector.tensor_scalar(
                out=dup[:], in0=dup[:], scalar1=1.0, scalar2=None, op0=OP.subtract
            )
            nc.vector.tensor_tensor(out=g1[:], in0=g1[:], in1=dup[:], op=OP.mult)
            nc.vector.tensor_scalar(
                out=g1[:], in0=g1[:], scalar1=-1.0, scalar2=None, op0=OP.mult
            )
            nc.vector.tensor_scalar(
                out=klf[:], in0=klf[:], scalar1=1.0, scalar2=None, op0=OP.add
            )
            nc.vector.tensor_tensor(out=klf[:], in0=klf[:], in1=g1[:], op=OP.mult)
            nc.vector.tensor_scalar(
                out=klf[:], in0=klf[:], scalar1=1.0, scalar2=None, op0=OP.subtract
            )
            kli = big.tile([128, 32 * W], i16)
            nc.vector.tensor_copy(kli[:], klf[:])

            mask_sb = big.tile([128, 32 * 512], bf16)
            for kc in range(32):
                nc.gpsimd.local_scatter(
                    out_ap=mask_sb[:, kc * 512 : (kc + 1) * 512],
                    data_ap=ones_w[:],
                    idxs_ap=kli[:, kc * W : (kc + 1) * W],
                    channels=128,
                    num_elems=QR,
                    num_idxs=W,
                )

            # ---------- x -> bf16 -> x^T (xbar transpose) ----------
            for chn in range(8):
                xs = work.tile([128, 1024], f32, tag="xstage")
                nc.sync.dma_start(
                    out=xs[:].rearrange("p (b c) -> p b c", b=4),
                    in_=x_full[chn * 512 : (chn + 1) * 512, :].rearrange(
                        "(b p) c -> p b c", p=128
                    ),
                )
                xb = work.tile([128, 1024], bf16, tag="xbstage")
                nc.vector.tensor_copy(xb[:], xs[:])
                nc.sync.dma_start(
                    out=xbf_dram[chn * 512 : (chn + 1) * 512, :].rearrange(
                        "(b p) c -> p b c", p=128
                    ),
                    in_=xb[:].rearrange("p (b c) -> p b c", b=4),
                )
            xT = [big.tile([128, N], bf16, tag=f"xT{i}", name=f"xT{i}") for i in range(2)]
            for cb in range(2):
                nc.sync.dma_start_transpose(
                    out=xT[cb][:], in_=xbf_dram[:, cb * 128 : (cb + 1) * 128]
                )

            # ---------- W^T via PE transposes; x_rows^T ----------
            WqT = [cons.tile([128, D], bf16, tag=f"wqt{i}", name=f"wqt{i}") for i in range(2)]
            WkT = [cons.tile([128, D], bf16, tag=f"wkt{i}", name=f"wkt{i}") for i in range(2)]
            WvT = [cons.tile([128, D], bf16, tag=f"wvt{i}", name=f"wvt{i}") for i in range(2)]
            WpT = [cons.tile([128, D], f32, tag=f"wpt{i}", name=f"wpt{i}") for i in range(4)]
            xrT_f = [cons.tile([128, QR], f32, tag=f"xrtf{i}", name=f"xrtf{i}") for i in range(2)]
            xrT_b = [cons.tile([128, QR], bf16, tag=f"xrtb{i}", name=f"xrtb{i}") for i in range(2)]

            with tc.tile_pool(name="pst", bufs=2, space="PSUM") as pst:
                def transpose_to(dst_tiles, Wt, rows, cols, extra=None):
                    wsb = [
                        work.tile(
                            [128, cols], f32, tag=f"wload{rb}", name=f"wload{rb}",
                            bufs=1,
                        )
                        for rb in range(rows // 128)
                    ]
                    for rb in range(rows // 128):
                        nc.sync.dma_start(
                            out=wsb[rb][:], in_=Wt[rb * 128 : (rb + 1) * 128, :]
                        )
                    for cb in range(cols // 128):
                        for rb in range(rows // 128):
                            tp = pst.tile([128, 128], f32, tag="tp")
                            nc.tensor.transpose(
                                out=tp[:],
                                in_=wsb[rb][:, cb * 128 : (cb + 1) * 128],
                                identity=ident[:],
                            )
                            nc.vector.tensor_copy(
                                dst_tiles[cb][:, rb * 128 : (rb + 1) * 128], tp[:]
                            )
                            if extra is not None:
                                nc.vector.tensor_copy(
                                    extra[cb][:, rb * 128 : (rb + 1) * 128], tp[:]
                                )

                transpose_to(WqT, Wq, D, D)
                transpose_to(WkT, Wk, D, D)
                transpose_to(WvT, Wv, D, D)
                transpose_to(WpT, Wp, D, 2 * D)
                transpose_to(xrT_f, x_rows, QR, D, extra=xrT_b)

            # ---------- projections (bf16) ----------
            QT = [big.tile([128, QR], bf16, tag=f"QT{i}", name=f"QT{i}") for i in range(2)]
            KT = [big.tile([128, N], bf16, tag=f"KT{i}", name=f"KT{i}") for i in range(2)]
            Vaug = big.tile([128, 32 * 264], bf16)
            nc.vector.memset(
                Vaug[:].rearrange("p (n h w) -> p n h w", n=32, h=8)[:, :, :, 32:33],
                1.0,
            )

            with tc.tile_pool(name="psp", bufs=2, space="PSUM") as psp:
                for dt_ in range(2):
                    qp = psp.tile([128, QR], f32, tag="qp")
                    for cc in range(2):
                        nc.tensor.matmul(
                            qp[:],
                            lhsT=WqT[cc][:, dt_ * 128 : (dt_ + 1) * 128],
                            rhs=xrT_b[cc][:],
                            start=(cc == 0),
                            stop=(cc == 1),
                        )
                    nc.vector.tensor_scalar(
                        out=QT[dt_][:], in0=qp[:], scalar1=bq_sb[:, dt_ : dt_ + 1],
                        scalar2=None, op0=OP.add,
                    )
                for dt_ in range(2):
                    for nch in range(8):
                        kp = psp.tile([128, 512], f32, tag="kp")
                        for cc in range(2):
                            nc.tensor.matmul(
                                kp[:],
                                lhsT=WkT[cc][:, dt_ * 128 : (dt_ + 1) * 128],
                                rhs=xT[cc][:, nch * 512 : (nch + 1) * 512],
                                start=(cc == 0),
                                stop=(cc == 1),
                            )
                        nc.vector.tensor_scalar(
                            out=KT[dt_][:, nch * 512 : (nch + 1) * 512], in0=kp[:],
                            scalar1=bk_sb[:, dt_ : dt_ + 1], scalar2=None, op0=OP.add,
                        )
                for nb in range(32):
                    vp = psp.tile([128, D], f32, tag="vp")
                    for cc in range(2):
                        nc.tensor.matmul(
                            vp[:],
                            lhsT=xT[cc][:, nb * 128 : (nb + 1) * 128],
                            rhs=WvT[cc][:],
                            start=(cc == 0),
                            stop=False,
                        )
                    nc.tensor.matmul(
                        vp[:], lhsT=ones_bf_1x128[:], rhs=bv_bf[:],
                        start=False, stop=True,
                    )
                    nc.vector.tensor_copy(
                        Vaug[:, nb * 264 : (nb + 1) * 264].rearrange(
                            "p (h w) -> p h w", w=33
                        )[:, :, 0:32],
                        vp[:].rearrange("p (h w) -> p h w", h=8),
                    )

            # ---------- attention ----------
            yT = [cons.tile([128, QR], f32, tag=f"yt{i}", name=f"yt{i}") for i in range(2)]
            yu = [
                cons.tile([32, QR], f32, tag=f"yu{h}", name=f"yu{h}")
                for h in range(H)
            ]
            zpack = cons.tile([H, QR], f32)
            rz8 = cons.tile([H, QR], f32)
            # SEL[k, h*32+m] = 1 if k == h: one-hot lhsT columns for the
            # per-head 1/Z row broadcasts
            sel = cons.tile([H, H * 32], f32)
            nc.sync.dma_start(out=sel[:], in_=sel_in[:])
            with tc.tile_pool(name="psa", bufs=1, space="PSUM") as psa:
                for g2 in range(4):
                    heads = (2 * g2, 2 * g2 + 1)
                    pv = [
                        psa.tile([33, 512], f32, tag=f"pv{j}", bufs=1, name=f"pv{j}")
                        for j in range(2)
                    ]
                    for kc in range(32):
                        sp = psa.tile([128, 1024], f32, tag="sp", bufs=2)
                        for j, h in enumerate(heads):
                            dt_, band = h // 4, (h % 4) * 32
                            nc.tensor.matmul(
                                sp[:, j * 512 : (j + 1) * 512],
                                lhsT=KT[dt_][band : band + 32, kc * 128 : (kc + 1) * 128],
                                rhs=QT[dt_][band : band + 32, :],
                                start=True,
                                stop=True,
                                tile_position=(band, 0),
                            )
                        praw = work.tile([128, 1024], bf16, tag="praw")
                        nc.scalar.activation(praw[:], sp[:], AF.Exp, scale=SCALE)
                        phat = work.tile([128, 1024], bf16, tag="phat")
                        for j in range(2):
                            nc.vector.tensor_tensor(
                                out=phat[:, j * 512 : (j + 1) * 512],
                                in0=praw[:, j * 512 : (j + 1) * 512],
                                in1=mask_sb[:, kc * 512 : (kc + 1) * 512],
                                op=OP.mult,
                            )
                        for j, h in enumerate(heads):
                            nc.tensor.matmul(
                                pv[j][:],
                                lhsT=Vaug[:, (kc * 8 + h) * 33 : (kc * 8 + h + 1) * 33],
                                rhs=phat[:, j * 512 : (j + 1) * 512],
                                start=(kc == 0),
                                stop=(kc == 31),
                            )
                    for j, h in enumerate(heads):
                        nc.vector.tensor_copy(yu[h][:], pv[j][0:32, :])
                        zt = work.tile([33, 512], f32, tag="zt")
                        nc.vector.tensor_copy(zt[32:33, :], pv[j][32:33, :])
                        nc.sync.dma_start(
                            out=zpack[h : h + 1, :], in_=zt[32:33, :]
                        )

                # softmax denominators: one batched reciprocal, then per-head
                # row broadcast via one-hot PE matmuls (all partition-aligned)
                nc.vector.reciprocal(rz8[:], zpack[:])
                for h in range(H):
                    dt_, band = h // 4, (h % 4) * 32
                    rp = psa.tile([32, 512], f32, tag="rp", bufs=2)
                    nc.tensor.matmul(
                        rp[:],
                        lhsT=sel[:, h * 32 : (h + 1) * 32],
                        rhs=rz8[:],
                        start=True,
                        stop=True,
                    )
                    rsb = work.tile([32, 512], f32, tag="rsb")
                    nc.vector.tensor_copy(rsb[:], rp[:])
                    ys = work.tile([32, 512], f32, tag="ys")
                    nc.vector.tensor_tensor(
                        out=ys[:], in0=yu[h][:], in1=rsb[:], op=OP.mult
                    )
                    nc.sync.dma_start(out=yT[dt_][band : band + 32, :], in_=ys[:])

            # ---------- final projection (fp32) ----------
            catT = [xrT_f[0], xrT_f[1], yT[0], yT[1]]
            with tc.tile_pool(name="pso", bufs=2, space="PSUM") as pso:
                for qb in range(4):
                    op_ = pso.tile([128, D], f32, tag="op")
                    for cc in range(4):
                        nc.tensor.matmul(
                            op_[:],
                            lhsT=catT[cc][:, qb * 128 : (qb + 1) * 128],
                            rhs=WpT[cc][:],
                            start=(cc == 0),
                            stop=False,
                        )
                    nc.tensor.matmul(
                        op_[:],
                        lhsT=ones_f_128x128[0:1, :],
                        rhs=bp_sb[:],
                        start=False,
                        stop=True,
                    )
                    osb = work.tile([128, D], f32, tag="osb")
                    nc.vector.tensor_copy(osb[:], op_[:])
                    nc.sync.dma_start(
                        out=out[qb * 128 : (qb + 1) * 128, :], in_=osb[:]
                    )

    lower_extended_insts(nc)
    if split:
        _split_multi_waits(nc)
    return nc


_PROGRAM = None


def _get_program():
    global _PROGRAM
    if _PROGRAM is None:
        _PROGRAM = build_program()
    return _PROGRAM


def shard_inputs(inputs):
    x = np.ascontiguousarray(np.asarray(inputs["x"], dtype=np.float32))
    ei = np.asarray(inputs["edge_index"])
    src = ei[0].astype(np.int64)
    dst = ei[1].astype(np.int64)
    Wq_ = np.ascontiguousarray(np.asarray(inputs["Wq"], np.float32))
    Wk_ = np.ascontiguousarray(np.asarray(inputs["Wk"], np.float32))
    Wv_ = np.ascontiguousarray(np.asarray(inputs["Wv"], np.float32))
    Wp_ = np.ascontiguousarray(np.asarray(inputs["Wp"], np.float32))
    bq_ = np.ascontiguousarray(np.asarray(inputs["bq"], np.float32).reshape(D, 1))
    bk_ = np.ascontiguousarray(np.asarray(inputs["bk"], np.float32).reshape(D, 1))
    bv_ = np.ascontiguousarray(np.asarray(inputs["bv"], np.float32).reshape(1, D))
    bp_ = np.ascontiguousarray(np.asarray(inputs["bp"], np.float32).reshape(1, D))
    ident = np.eye(128, dtype=np.float32)
    selmat = np.zeros((H, H * 32), np.float32)
    for h in range(H):
        selmat[h, h * 32 : (h + 1) * 32] = 1.0

    in_maps = []
    for c in range(NCORES):
        q0 = c * QR
        sel = (src >= q0) & (src < q0 + QR)
        es, ed = src[sel], dst[sel]
        # group the shard by key; sort so duplicate edges are adjacent
        order = np.argsort(ed * N + es)
        es, ed = es[order], ed[order]
        counts = np.bincount(ed, minlength=N)
        if counts.max() > W:
            raise ValueError(f"per-key list overflow: {counts.max()} > {W}")
        starts = np.concatenate([[0], np.cumsum(counts)[:-1]])
        ranks = np.arange(len(ed)) - starts[ed]
        lists = np.full((N, W), -1, np.int16)
        lists[ed, ranks] = es.astype(np.int16)
        in_maps.append(
            {
                "x_full": x,
                "x_rows": np.ascontiguousarray(x[q0 : q0 + QR]),
                "klists": lists,
                "q0v": np.full((128, 1), float(q0), np.float32),
                "ident_in": ident,
                "sel_in": selmat,
                "Wq": Wq_, "Wk": Wk_, "Wv": Wv_, "Wp": Wp_,
                "bq": bq_, "bk": bk_, "bv": bv_, "bp": bp_,
            }
        )
    return in_maps


def run(inputs, trace=False):
    from concourse.bass_utils import run_bass_kernel_spmd

    nc = _get_program()
    in_maps = shard_inputs(inputs)
    res = run_bass_kernel_spmd(nc, in_maps, core_ids=list(range(NCORES)), trace=trace)
    full = np.concatenate([res.results[c]["out"] for c in range(NCORES)], axis=0)
    return np.ascontiguousarray(full.astype(np.float32)), res


def kernel(**inputs) -> np.ndarray:
    out, _ = run(inputs, trace=False)
    return out
